# revision 1
# baseline (speedup 1.0000x reference)
"""GRUAggregation1d Trainium2 kernel.

Computes, for xs [B=16, 512, L=8192], z_prev [B, 128, L] (all fp32):
    q  = sigmoid(Wq@xs + Uq@z + bq)        (per position l, batch b)
    r  = sigmoid(Wr@xs + Ur@z + br)
    zt = tanh(Wz@xs + Uz@(r*z) + bz)
    out = q*z + (1-q)*zt

Sharding: data-parallel over batch. 8 cores x 2 batches each; weights
replicated. Each core loops over 2 batches x 16 position-tiles of 512.
Per tile: 15 matmuls (3 gates x (4 K-chunks of W + 1 U matmul)) accumulated
in PSUM, sigmoid/tanh on ScalarE (bias fused), gate combine on VectorE.
One-iteration software pipeline: the Uz@(r*z) matmul + tanh + combine of
tile i are emitted during tile i+1 so the PE never stalls on the
r -> r*z dependency chain.
"""

from contextlib import ExitStack

import numpy as np

import concourse.bass as bass
import concourse.mybir as mybir
import concourse.tile as tile
from concourse import bacc
from concourse.bass_utils import run_bass_kernel_spmd

B, IN_DIM, WIDTH, L = 16, 512, 128, 8192
N_CORES = 8
B_PER = B // N_CORES          # batches per core
KC = IN_DIM // 128            # K chunks for the W matmuls
NT = 512                      # positions per tile
N_LT = L // NT                # position tiles per batch
F32 = mybir.dt.float32

# matmul input interpretation: "f32" (exact, 4 cyc/row), "f32r" (fast fp32,
# 1 cyc/row at N>=256, reduced internal precision)
MM_DTYPE = "f32"


def _mm_ap(ap):
    if MM_DTYPE == "f32r":
        return ap.bitcast(mybir.dt.float32r)
    return ap


_module_cache = {}


def _build():
    key = (MM_DTYPE, NT)
    if key in _module_cache:
        return _module_cache[key]

    nc = bacc.Bacc("TRN2", target_bir_lowering=False, debug=False,
                   num_devices=N_CORES)

    xs_d = nc.dram_tensor("xs", [B_PER, IN_DIM, L], F32, kind="ExternalInput").ap()
    zp_d = nc.dram_tensor("zp", [B_PER, WIDTH, L], F32, kind="ExternalInput").ap()
    w_d = {}
    for g in ("q", "r", "z"):
        w_d[f"w{g}"] = nc.dram_tensor(f"w{g}", [128, KC, 128], F32,
                                      kind="ExternalInput").ap()
        w_d[f"u{g}"] = nc.dram_tensor(f"u{g}", [128, 128], F32,
                                      kind="ExternalInput").ap()
        w_d[f"b{g}"] = nc.dram_tensor(f"b{g}", [128, 1], F32,
                                      kind="ExternalInput").ap()
    out_d = nc.dram_tensor("out", [B_PER, WIDTH, L], F32, kind="ExternalOutput").ap()

    # [b, (k p), l] -> [b, p, k, l] so a position-tile slice is a [128, KC, NT]
    # DMA with 2KB contiguous rows
    xs_r = xs_d.rearrange("b (k p) l -> b p k l", p=128)

    with tile.TileContext(nc) as tc, ExitStack() as ctx:
        wpool = ctx.enter_context(tc.tile_pool(name="weights", bufs=1))
        io = ctx.enter_context(tc.tile_pool(name="io", bufs=3))
        acts = ctx.enter_context(tc.tile_pool(name="acts", bufs=3))
        psum = ctx.enter_context(tc.tile_pool(name="psum", bufs=2, space="PSUM"))

        # weights + biases, loaded once
        w_sb = {}
        for g in ("q", "r", "z"):
            wt = wpool.tile([128, KC, 128], F32, tag=f"w{g}")
            nc.sync.dma_start(wt[:], w_d[f"w{g}"][:])
            ut = wpool.tile([128, 128], F32, tag=f"u{g}")
            nc.sync.dma_start(ut[:], w_d[f"u{g}"][:])
            bt = wpool.tile([128, 1], F32, tag=f"b{g}")
            nc.sync.dma_start(bt[:], w_d[f"b{g}"][:])
            w_sb[g] = (wt, ut, bt)

        # one software-pipeline stage of carried state per tile:
        # (zt_psum, q_sbuf, z_sbuf, out_slice)
        carry = None

        def finish(carry):
            zt_ps, q_s, z_t, out_slice = carry
            _, _, bz_t = w_sb["z"]
            zt_s = acts.tile([128, NT], F32, tag="zt_s")
            nc.scalar.activation(zt_s[:], zt_ps[:],
                                 mybir.ActivationFunctionType.Tanh, bias=bz_t[:])
            # out = zt + q*(z - zt)
            diff = acts.tile([128, NT], F32, tag="diff")
            nc.vector.tensor_sub(diff[:], z_t[:], zt_s[:])
            prod = acts.tile([128, NT], F32, tag="prod")
            nc.vector.tensor_mul(prod[:], q_s[:], diff[:])
            o_t = acts.tile([128, NT], F32, tag="o_t")
            nc.vector.tensor_add(o_t[:], zt_s[:], prod[:])
            nc.sync.dma_start(out_slice, o_t[:])

        for b in range(B_PER):
            for i in range(N_LT):
                l0 = i * NT
                xs_t = io.tile([128, KC, NT], F32, tag="xs_t")
                nc.sync.dma_start(xs_t[:], xs_r[b][:, :, l0:l0 + NT])
                z_t = io.tile([128, NT], F32, tag="z_t")
                nc.sync.dma_start(z_t[:], zp_d[b][:, l0:l0 + NT])

                # ---- PE: finish previous tile's zt accumulation first ----
                if carry is not None:
                    prev_zt_ps, _, prev_z, _ = carry
                    _, uz_t, _ = w_sb["z"]
                    nc.tensor.matmul(prev_zt_ps[:], _mm_ap(uz_t[:]),
                                     _mm_ap(carry_rz[:]), start=False, stop=True)
                    finish(carry)
                    carry = None

                # ---- q gate ----
                wq_t, uq_t, bq_t = w_sb["q"]
                q_ps = psum.tile([128, NT], F32, tag="q_ps")
                for k in range(KC):
                    nc.tensor.matmul(q_ps[:], _mm_ap(wq_t[:, k, :]),
                                     _mm_ap(xs_t[:, k, :]), start=(k == 0),
                                     stop=False)
                nc.tensor.matmul(q_ps[:], _mm_ap(uq_t[:]), _mm_ap(z_t[:]),
                                 start=False, stop=True)
                q_s = acts.tile([128, NT], F32, tag="q_s")
                nc.scalar.activation(q_s[:], q_ps[:],
                                     mybir.ActivationFunctionType.Sigmoid,
                                     bias=bq_t[:])

                # ---- r gate ----
                wr_t, ur_t, br_t = w_sb["r"]
                r_ps = psum.tile([128, NT], F32, tag="r_ps")
                for k in range(KC):
                    nc.tensor.matmul(r_ps[:], _mm_ap(wr_t[:, k, :]),
                                     _mm_ap(xs_t[:, k, :]), start=(k == 0),
                                     stop=False)
                nc.tensor.matmul(r_ps[:], _mm_ap(ur_t[:]), _mm_ap(z_t[:]),
                                 start=False, stop=True)
                r_s = acts.tile([128, NT], F32, tag="r_s")
                nc.scalar.activation(r_s[:], r_ps[:],
                                     mybir.ActivationFunctionType.Sigmoid,
                                     bias=br_t[:])

                # ---- zt: W part only; Uz@(r*z) lands next iteration ----
                wz_t, _, _ = w_sb["z"]
                zt_ps = psum.tile([128, NT], F32, tag="zt_ps")
                for k in range(KC):
                    nc.tensor.matmul(zt_ps[:], _mm_ap(wz_t[:, k, :]),
                                     _mm_ap(xs_t[:, k, :]), start=(k == 0),
                                     stop=False)

                rz = acts.tile([128, NT], F32, tag="rz")
                nc.vector.tensor_mul(rz[:], r_s[:], z_t[:])
                carry_rz = rz
                carry = (zt_ps, q_s, z_t, out_d[b][:, l0:l0 + NT])

        # drain the last tile
        prev_zt_ps, _, _, _ = carry
        _, uz_t, _ = w_sb["z"]
        nc.tensor.matmul(prev_zt_ps[:], _mm_ap(uz_t[:]), _mm_ap(carry_rz[:]),
                         start=False, stop=True)
        finish(carry)

    nc.compile()
    _module_cache[key] = nc
    return nc


def _pack_w(w):
    # W [128 out, 512 in] -> [128 part=in%128, KC, 128 out]
    return np.ascontiguousarray(
        w.T.reshape(KC, 128, 128).transpose(1, 0, 2)).astype(np.float32)


def _run(inputs, trace=False, **run_kwargs):
    xs = np.ascontiguousarray(np.asarray(inputs["xs"], dtype=np.float32))
    zp = np.ascontiguousarray(np.asarray(inputs["z_prev"], dtype=np.float32))
    assert xs.shape == (B, IN_DIM, L) and zp.shape == (B, WIDTH, L)

    packed = {}
    for g, (wn, un, wbn, ubn) in {
        "q": ("Wq_w", "Uq_w", "Wq_b", "Uq_b"),
        "r": ("Wr_w", "Ur_w", "Wr_b", "Ur_b"),
        "z": ("Wz_w", "Uz_w", "Wz_b", "Uz_b"),
    }.items():
        packed[f"w{g}"] = _pack_w(np.asarray(inputs[wn], dtype=np.float32))
        packed[f"u{g}"] = np.ascontiguousarray(
            np.asarray(inputs[un], dtype=np.float32).T)
        packed[f"b{g}"] = np.ascontiguousarray(
            (np.asarray(inputs[wbn], dtype=np.float32)
             + np.asarray(inputs[ubn], dtype=np.float32)).reshape(128, 1))

    nc = _build()
    in_maps = []
    for c in range(N_CORES):
        m = {"xs": np.ascontiguousarray(xs[c * B_PER:(c + 1) * B_PER]),
             "zp": np.ascontiguousarray(zp[c * B_PER:(c + 1) * B_PER])}
        m.update(packed)
        in_maps.append(m)

    res = run_bass_kernel_spmd(nc, in_maps, core_ids=list(range(N_CORES)),
                               trace=trace, **run_kwargs)
    out = np.concatenate([res.results[c]["out"] for c in range(N_CORES)], axis=0)
    return out, res


def kernel(**inputs):
    out, _ = _run(inputs, trace=False)
    return out


# revision 4
# speedup vs baseline: 1.9654x; 1.9654x over previous
"""GRUAggregation1d Trainium2 kernel.

Computes, for xs [B=16, 512, L=8192], z_prev [B, 128, L] (all fp32):
    q  = sigmoid(Wq@xs + Uq@z + bq)        (per position l, batch b)
    r  = sigmoid(Wr@xs + Ur@z + br)
    zt = tanh(Wz@xs + Uz@(r*z) + bz)
    out = q*z + (1-q)*zt

Sharding: data-parallel over batch. 8 cores x 2 batches each; weights
replicated. Each core loops over 2 batches x 16 position-tiles of 512.
Per tile: 15 matmuls (3 gates x (4 K-chunks of W + 1 U matmul)) accumulated
in PSUM, sigmoid/tanh on ScalarE (bias fused), gate combine on VectorE.
One-iteration software pipeline: the Uz@(r*z) matmul + tanh + combine of
tile i are emitted during tile i+1 so the PE never stalls on the
r -> r*z dependency chain.
"""

from contextlib import ExitStack

import numpy as np

import concourse.bass as bass
import concourse.mybir as mybir
import concourse.tile as tile
from concourse import bacc
from concourse.bass_utils import run_bass_kernel_spmd

B, IN_DIM, WIDTH, L = 16, 512, 128, 8192
N_CORES = 8
B_PER = B // N_CORES          # batches per core
KC = IN_DIM // 128            # K chunks for the W matmuls
NT = 512                      # positions per tile
N_LT = L // NT                # position tiles per batch
F32 = mybir.dt.float32

# matmul input interpretation: "f32" (exact, 4 cyc/row), "f32r" (fast fp32,
# 1 cyc/row at N>=256, reduced internal precision)
MM_DTYPE = "f32r"


def _mm_dt():
    return mybir.dt.float32r if MM_DTYPE == "f32r" else F32


def _mm_ap(ap):
    return ap


_module_cache = {}


def _build():
    key = (MM_DTYPE, NT)
    if key in _module_cache:
        return _module_cache[key]

    nc = bacc.Bacc("TRN2", target_bir_lowering=False, debug=False,
                   num_devices=N_CORES)

    mmdt = _mm_dt()
    xs_d = nc.dram_tensor("xs", [B_PER, IN_DIM, L], mmdt, kind="ExternalInput").ap()
    zp_d = nc.dram_tensor("zp", [B_PER, WIDTH, L], mmdt, kind="ExternalInput").ap()
    w_d = {}
    for g in ("q", "r", "z"):
        w_d[f"w{g}"] = nc.dram_tensor(f"w{g}", [128, KC, 128], mmdt,
                                      kind="ExternalInput").ap()
        w_d[f"u{g}"] = nc.dram_tensor(f"u{g}", [128, 128], mmdt,
                                      kind="ExternalInput").ap()
        w_d[f"b{g}"] = nc.dram_tensor(f"b{g}", [128, 1], F32,
                                      kind="ExternalInput").ap()
    out_d = nc.dram_tensor("out", [B_PER, WIDTH, L], F32, kind="ExternalOutput").ap()

    # [b, (k p), l] -> [b, p, k, l] so a position-tile slice is a [128, KC, NT]
    # DMA with 2KB contiguous rows
    xs_r = xs_d.rearrange("b (k p) l -> b p k l", p=128)

    with tile.TileContext(nc) as tc, ExitStack() as ctx:
        wpool = ctx.enter_context(tc.tile_pool(name="weights", bufs=1))
        io = ctx.enter_context(tc.tile_pool(name="io", bufs=3))
        acts = ctx.enter_context(tc.tile_pool(name="acts", bufs=3))
        psum = ctx.enter_context(tc.tile_pool(name="psum", bufs=2, space="PSUM"))

        # weights + biases, loaded once
        w_sb = {}
        for g in ("q", "r", "z"):
            wt = wpool.tile([128, KC, 128], mmdt, tag=f"w{g}")
            nc.sync.dma_start(wt[:], w_d[f"w{g}"][:])
            ut = wpool.tile([128, 128], mmdt, tag=f"u{g}")
            nc.sync.dma_start(ut[:], w_d[f"u{g}"][:])
            bt = wpool.tile([128, 1], F32, tag=f"b{g}")
            nc.sync.dma_start(bt[:], w_d[f"b{g}"][:])
            w_sb[g] = (wt, ut, bt)

        # one software-pipeline stage of carried state per tile:
        # (zt_psum, q_sbuf, z_sbuf, out_slice)
        carry = None

        def finish(carry):
            zt_ps, q_s, z_t, out_slice = carry
            _, _, bz_t = w_sb["z"]
            zt_s = acts.tile([128, NT], F32, tag="zt_s")
            nc.scalar.activation(zt_s[:], zt_ps[:],
                                 mybir.ActivationFunctionType.Tanh, bias=bz_t[:])
            # out = zt + q*(z - zt)
            diff = acts.tile([128, NT], F32, tag="diff")
            nc.vector.tensor_sub(diff[:], z_t[:], zt_s[:])
            prod = acts.tile([128, NT], F32, tag="prod")
            nc.vector.tensor_mul(prod[:], q_s[:], diff[:])
            o_t = acts.tile([128, NT], F32, tag="o_t")
            nc.vector.tensor_add(o_t[:], zt_s[:], prod[:])
            nc.sync.dma_start(out_slice, o_t[:])

        for b in range(B_PER):
            for i in range(N_LT):
                l0 = i * NT
                xs_t = io.tile([128, KC, NT], mmdt, tag="xs_t")
                nc.sync.dma_start(xs_t[:], xs_r[b][:, :, l0:l0 + NT])
                z_t = io.tile([128, NT], mmdt, tag="z_t")
                nc.sync.dma_start(z_t[:], zp_d[b][:, l0:l0 + NT])

                # ---- PE: finish previous tile's zt accumulation first ----
                if carry is not None:
                    prev_zt_ps, _, prev_z, _ = carry
                    _, uz_t, _ = w_sb["z"]
                    nc.tensor.matmul(prev_zt_ps[:], _mm_ap(uz_t[:]),
                                     _mm_ap(carry_rz[:]), start=False, stop=True)
                    finish(carry)
                    carry = None

                # ---- q gate ----
                wq_t, uq_t, bq_t = w_sb["q"]
                q_ps = psum.tile([128, NT], F32, tag="q_ps")
                for k in range(KC):
                    nc.tensor.matmul(q_ps[:], _mm_ap(wq_t[:, k, :]),
                                     _mm_ap(xs_t[:, k, :]), start=(k == 0),
                                     stop=False)
                nc.tensor.matmul(q_ps[:], _mm_ap(uq_t[:]), _mm_ap(z_t[:]),
                                 start=False, stop=True)
                q_s = acts.tile([128, NT], F32, tag="q_s")
                nc.scalar.activation(q_s[:], q_ps[:],
                                     mybir.ActivationFunctionType.Sigmoid,
                                     bias=bq_t[:])

                # ---- r gate ----
                wr_t, ur_t, br_t = w_sb["r"]
                r_ps = psum.tile([128, NT], F32, tag="r_ps")
                for k in range(KC):
                    nc.tensor.matmul(r_ps[:], _mm_ap(wr_t[:, k, :]),
                                     _mm_ap(xs_t[:, k, :]), start=(k == 0),
                                     stop=False)
                nc.tensor.matmul(r_ps[:], _mm_ap(ur_t[:]), _mm_ap(z_t[:]),
                                 start=False, stop=True)
                r_s = acts.tile([128, NT], F32, tag="r_s")
                nc.scalar.activation(r_s[:], r_ps[:],
                                     mybir.ActivationFunctionType.Sigmoid,
                                     bias=br_t[:])

                # ---- zt: W part only; Uz@(r*z) lands next iteration ----
                wz_t, _, _ = w_sb["z"]
                zt_ps = psum.tile([128, NT], F32, tag="zt_ps")
                for k in range(KC):
                    nc.tensor.matmul(zt_ps[:], _mm_ap(wz_t[:, k, :]),
                                     _mm_ap(xs_t[:, k, :]), start=(k == 0),
                                     stop=False)

                rz = acts.tile([128, NT], mmdt, tag="rz")
                nc.vector.tensor_mul(rz[:], r_s[:], z_t[:])
                carry_rz = rz
                carry = (zt_ps, q_s, z_t, out_d[b][:, l0:l0 + NT])

        # drain the last tile
        prev_zt_ps, _, _, _ = carry
        _, uz_t, _ = w_sb["z"]
        nc.tensor.matmul(prev_zt_ps[:], _mm_ap(uz_t[:]), _mm_ap(carry_rz[:]),
                         start=False, stop=True)
        finish(carry)

    nc.compile()
    _module_cache[key] = nc
    return nc


def _pack_w(w):
    # W [128 out, 512 in] -> [128 part=in%128, KC, 128 out]
    return np.ascontiguousarray(
        w.T.reshape(KC, 128, 128).transpose(1, 0, 2)).astype(np.float32)


def _run(inputs, trace=False, **run_kwargs):
    xs = np.ascontiguousarray(np.asarray(inputs["xs"], dtype=np.float32))
    zp = np.ascontiguousarray(np.asarray(inputs["z_prev"], dtype=np.float32))
    assert xs.shape == (B, IN_DIM, L) and zp.shape == (B, WIDTH, L)

    packed = {}
    for g, (wn, un, wbn, ubn) in {
        "q": ("Wq_w", "Uq_w", "Wq_b", "Uq_b"),
        "r": ("Wr_w", "Ur_w", "Wr_b", "Ur_b"),
        "z": ("Wz_w", "Uz_w", "Wz_b", "Uz_b"),
    }.items():
        packed[f"w{g}"] = _pack_w(np.asarray(inputs[wn], dtype=np.float32))
        packed[f"u{g}"] = np.ascontiguousarray(
            np.asarray(inputs[un], dtype=np.float32).T)
        packed[f"b{g}"] = np.ascontiguousarray(
            (np.asarray(inputs[wbn], dtype=np.float32)
             + np.asarray(inputs[ubn], dtype=np.float32)).reshape(128, 1))

    nc = _build()
    in_maps = []
    for c in range(N_CORES):
        m = {"xs": np.ascontiguousarray(xs[c * B_PER:(c + 1) * B_PER]),
             "zp": np.ascontiguousarray(zp[c * B_PER:(c + 1) * B_PER])}
        m.update(packed)
        in_maps.append(m)

    res = run_bass_kernel_spmd(nc, in_maps, core_ids=list(range(N_CORES)),
                               trace=trace, **run_kwargs)
    out = np.concatenate([res.results[c]["out"] for c in range(N_CORES)], axis=0)
    return out, res


def kernel(**inputs):
    out, _ = _run(inputs, trace=False)
    return out


# revision 5
# speedup vs baseline: 2.7581x; 1.4033x over previous
"""GRUAggregation1d Trainium2 kernel.

Computes, for xs [B=16, 512, L=8192], z_prev [B, 128, L] (all fp32):
    q  = sigmoid(Wq@xs + Uq@z + bq)        (per position l, batch b)
    r  = sigmoid(Wr@xs + Ur@z + br)
    zt = tanh(Wz@xs + Uz@(r*z) + bz)
    out = q*z + (1-q)*zt

Sharding: data-parallel over batch. 8 cores x 2 batches each; weights
replicated. Each core loops over 2 batches x 16 position-tiles of 512.
Per tile: 15 matmuls (3 gates x (4 K-chunks of W + 1 U matmul)) accumulated
in PSUM, sigmoid/tanh on ScalarE (bias fused), gate combine on VectorE.
One-iteration software pipeline: the Uz@(r*z) matmul + tanh + combine of
tile i are emitted during tile i+1 so the PE never stalls on the
r -> r*z dependency chain.

Matmul inputs are bf16 (xs and the weights are cast on the host, halving
the xs DMA; z_prev is DMA'd fp32 and cast to bf16 on ScalarE so the final
combine q*z + (1-q)*zt still sees fp32 z). PSUM accumulation is fp32.
"""

from contextlib import ExitStack

import ml_dtypes
import numpy as np

import concourse.bass as bass
import concourse.mybir as mybir
import concourse.tile as tile
from concourse import bacc
from concourse.bass_utils import run_bass_kernel_spmd

B, IN_DIM, WIDTH, L = 16, 512, 128, 8192
N_CORES = 8
B_PER = B // N_CORES          # batches per core
KC = IN_DIM // 128            # K chunks for the W matmuls
NT = 512                      # positions per tile
N_LT = L // NT                # position tiles per batch
F32 = mybir.dt.float32
BF16 = mybir.dt.bfloat16

_module_cache = {}


def _build():
    key = ("bf16", NT)
    if key in _module_cache:
        return _module_cache[key]

    nc = bacc.Bacc("TRN2", target_bir_lowering=False, debug=False,
                   num_devices=N_CORES)

    xs_d = nc.dram_tensor("xs", [B_PER, IN_DIM, L], BF16, kind="ExternalInput").ap()
    zp_d = nc.dram_tensor("zp", [B_PER, WIDTH, L], F32, kind="ExternalInput").ap()
    w_d = {}
    for g in ("q", "r", "z"):
        w_d[f"w{g}"] = nc.dram_tensor(f"w{g}", [128, KC, 128], BF16,
                                      kind="ExternalInput").ap()
        w_d[f"u{g}"] = nc.dram_tensor(f"u{g}", [128, 128], BF16,
                                      kind="ExternalInput").ap()
        w_d[f"b{g}"] = nc.dram_tensor(f"b{g}", [128, 1], F32,
                                      kind="ExternalInput").ap()
    out_d = nc.dram_tensor("out", [B_PER, WIDTH, L], F32, kind="ExternalOutput").ap()

    # [b, (k p), l] -> [b, p, k, l] so a position-tile slice is a [128, KC, NT]
    # DMA with 1KB contiguous rows
    xs_r = xs_d.rearrange("b (k p) l -> b p k l", p=128)

    with tile.TileContext(nc) as tc, ExitStack() as ctx:
        wpool = ctx.enter_context(tc.tile_pool(name="weights", bufs=1))
        io = ctx.enter_context(tc.tile_pool(name="io", bufs=3))
        acts = ctx.enter_context(tc.tile_pool(name="acts", bufs=3))
        psum = ctx.enter_context(tc.tile_pool(name="psum", bufs=2, space="PSUM"))

        # weights + biases, loaded once
        w_sb = {}
        for g in ("q", "r", "z"):
            wt = wpool.tile([128, KC, 128], BF16, tag=f"w{g}")
            nc.sync.dma_start(wt[:], w_d[f"w{g}"][:])
            ut = wpool.tile([128, 128], BF16, tag=f"u{g}")
            nc.sync.dma_start(ut[:], w_d[f"u{g}"][:])
            bt = wpool.tile([128, 1], F32, tag=f"b{g}")
            nc.sync.dma_start(bt[:], w_d[f"b{g}"][:])
            w_sb[g] = (wt, ut, bt)

        # one software-pipeline stage of carried state per tile:
        # (zt_psum, rz, q_sbuf, z_sbuf, out_slice)
        carry = None

        def finish_prev(carry):
            """Emit the trailing half of tile i (Uz matmul, tanh, combine,
            store) -- called while tile i+1's leading half is in flight."""
            zt_ps, rz, q_s, z_t, out_slice = carry
            _, uz_t, bz_t = w_sb["z"]
            nc.tensor.matmul(zt_ps[:], uz_t[:], rz[:], start=False, stop=True)
            zt_s = acts.tile([128, NT], F32, tag="zt_s")
            nc.scalar.activation(zt_s[:], zt_ps[:],
                                 mybir.ActivationFunctionType.Tanh, bias=bz_t[:])
            # out = zt + q*(z - zt)
            diff = acts.tile([128, NT], F32, tag="diff")
            nc.vector.tensor_sub(diff[:], z_t[:], zt_s[:])
            prod = acts.tile([128, NT], F32, tag="prod")
            nc.vector.tensor_mul(prod[:], q_s[:], diff[:])
            o_t = acts.tile([128, NT], F32, tag="o_t")
            nc.vector.tensor_add(o_t[:], zt_s[:], prod[:])
            nc.sync.dma_start(out_slice, o_t[:])

        for b in range(B_PER):
            for i in range(N_LT):
                l0 = i * NT
                xs_t = io.tile([128, KC, NT], BF16, tag="xs_t")
                nc.sync.dma_start(xs_t[:], xs_r[b][:, :, l0:l0 + NT])
                z_t = io.tile([128, NT], F32, tag="z_t")
                nc.sync.dma_start(z_t[:], zp_d[b][:, l0:l0 + NT])
                # bf16 copy of z for the U matmuls (ScalarE has spare cycles)
                z_bf = io.tile([128, NT], BF16, tag="z_bf")
                nc.scalar.activation(z_bf[:], z_t[:],
                                     mybir.ActivationFunctionType.Copy)

                if carry is not None:
                    finish_prev(carry)
                    carry = None

                # ---- q gate ----
                wq_t, uq_t, bq_t = w_sb["q"]
                q_ps = psum.tile([128, NT], F32, tag="q_ps")
                for k in range(KC):
                    nc.tensor.matmul(q_ps[:], wq_t[:, k, :], xs_t[:, k, :],
                                     start=(k == 0), stop=False)
                nc.tensor.matmul(q_ps[:], uq_t[:], z_bf[:], start=False, stop=True)
                q_s = acts.tile([128, NT], F32, tag="q_s")
                nc.scalar.activation(q_s[:], q_ps[:],
                                     mybir.ActivationFunctionType.Sigmoid,
                                     bias=bq_t[:])

                # ---- r gate ----
                wr_t, ur_t, br_t = w_sb["r"]
                r_ps = psum.tile([128, NT], F32, tag="r_ps")
                for k in range(KC):
                    nc.tensor.matmul(r_ps[:], wr_t[:, k, :], xs_t[:, k, :],
                                     start=(k == 0), stop=False)
                nc.tensor.matmul(r_ps[:], ur_t[:], z_bf[:], start=False, stop=True)
                r_s = acts.tile([128, NT], BF16, tag="r_s")
                nc.scalar.activation(r_s[:], r_ps[:],
                                     mybir.ActivationFunctionType.Sigmoid,
                                     bias=br_t[:])

                # ---- zt: W part only; Uz@(r*z) lands next iteration ----
                wz_t, _, _ = w_sb["z"]
                zt_ps = psum.tile([128, NT], F32, tag="zt_ps")
                for k in range(KC):
                    nc.tensor.matmul(zt_ps[:], wz_t[:, k, :], xs_t[:, k, :],
                                     start=(k == 0), stop=False)

                rz = acts.tile([128, NT], BF16, tag="rz")
                nc.vector.tensor_mul(rz[:], r_s[:], z_bf[:])
                carry = (zt_ps, rz, q_s, z_t, out_d[b][:, l0:l0 + NT])

        finish_prev(carry)

    nc.compile()
    _module_cache[key] = nc
    return nc


def _pack_w(w):
    # W [128 out, 512 in] -> [128 part=in%128, KC, 128 out]
    return np.ascontiguousarray(
        w.T.reshape(KC, 128, 128).transpose(1, 0, 2)).astype(ml_dtypes.bfloat16)


def _run(inputs, trace=False, **run_kwargs):
    xs = np.asarray(inputs["xs"], dtype=np.float32)
    zp = np.ascontiguousarray(np.asarray(inputs["z_prev"], dtype=np.float32))
    assert xs.shape == (B, IN_DIM, L) and zp.shape == (B, WIDTH, L)
    xs_bf = np.ascontiguousarray(xs.astype(ml_dtypes.bfloat16))

    packed = {}
    for g, (wn, un, wbn, ubn) in {
        "q": ("Wq_w", "Uq_w", "Wq_b", "Uq_b"),
        "r": ("Wr_w", "Ur_w", "Wr_b", "Ur_b"),
        "z": ("Wz_w", "Uz_w", "Wz_b", "Uz_b"),
    }.items():
        packed[f"w{g}"] = _pack_w(np.asarray(inputs[wn], dtype=np.float32))
        packed[f"u{g}"] = np.ascontiguousarray(
            np.asarray(inputs[un], dtype=np.float32).T.astype(ml_dtypes.bfloat16))
        packed[f"b{g}"] = np.ascontiguousarray(
            (np.asarray(inputs[wbn], dtype=np.float32)
             + np.asarray(inputs[ubn], dtype=np.float32)).reshape(128, 1))

    nc = _build()
    in_maps = []
    for c in range(N_CORES):
        m = {"xs": np.ascontiguousarray(xs_bf[c * B_PER:(c + 1) * B_PER]),
             "zp": np.ascontiguousarray(zp[c * B_PER:(c + 1) * B_PER])}
        m.update(packed)
        in_maps.append(m)

    res = run_bass_kernel_spmd(nc, in_maps, core_ids=list(range(N_CORES)),
                               trace=trace, **run_kwargs)
    out = np.concatenate([res.results[c]["out"] for c in range(N_CORES)], axis=0)
    return out, res


def kernel(**inputs):
    out, _ = _run(inputs, trace=False)
    return out
